# revision 7
# baseline (speedup 1.0000x reference)
"""Multi-head attention (B=2, S=4096, D=512, H=8) on 8 NeuronCores.

Sharding: data-parallel on batch x head-pair-parallel.  Core c handles
batch b = c//4 and heads (2*(c%4), 2*(c%4)+1).  Each core computes its
[4096, 128] slice of the output; the host scatters inputs / gathers
outputs.

Per-core kernel (Bass/Tile), operands in fp16 (fp32 PSUM accumulate).
The scalar engine -- exp over all 2*4096^2 scores at 1 elem/cycle/lane
@1.2GHz, ~265 us of ACTIVATE time -- is the hard bottleneck, so the
whole schedule exists to start it early and never let it starve:

  - t=0: dummy exp preloads the ACT spline tables (~2.7us) during the
    engine-startup barrier; ~3.4us of dummy matmuls trip the PE HAM
    clock-gate to 2.4GHz before the first projection.
  - DMA issue order is priority: tiny weight images (with the biases
    folded in as trailing f16 columns) first, then x columns [0:1024)
    as one 3D transfer, then the rest (the last piece is held behind
    the second by a deliberate 1-column WAW overlap, because the DMA
    engines round-robin fairly over everything outstanding).
  - Projections are software-pipelined INTO the attention sweep of the
    first two q-chunks, one K/Q block or V 128-token piece per exp
    chunk, each placed just after its x columns land and just before
    its consumer.
  - Q^T/K^T are [128(d of 2 heads), S]; consecutive h0/h1 S^T matmuls
    (K=64) co-run in different PE row groups.  V sits in natural [k,d]
    layout as [V_h | ones] 65-col weight tiles (the ones column makes
    the E@V matmul also emit softmax row sums).
  - exp chunks are [1024,1024,1536x20] per q-chunk (6 PSUM banks
    ping-ponged; ends on a big chunk so the next q-chunk's S^T matmuls
    are always covered); E@V is deferred through a 34-deep fp16 ring
    while the projection ring owns the last 2 PSUM banks, then popped
    from a FIFO at a per-chunk rate that keeps the in-order PE queue
    from ever head-blocking the ACT feed.
  - Output: transpose O^T via xbar DMA-transpose (PE transpose on the
    final q chunk, whose normalize is split across ACT+DVE), scale by
    the reciprocal row-sum, one 3D DMA per q-chunk to DRAM.

Measured on the 8 axon trn2 cores: ~288.5 us HW exec (vs 329 us
baseline), rel err ~9.2e-4 (ACTIVATE ~267us, >94% scalar-engine busy).
"""

import numpy as np

N_CORES = 8
S_FULL = 4096
D_MODEL = 512
HEAD = 64

# DVE exp offload: trailing columns of each exp chunk evaluated on the
# vector engine via a 2-instruction custom-DVE chain instead of the
# scalar engine (the bottleneck).  Keyed by chunk width; per phase.
WD_PRO = {1024: 192, 1536: 288}            # qc0/qc1 (DVE also runs projections)
WD_STE = {512: 128, 1024: 256, 1536: 384}  # qc2+

_cached = {}


def _register_exp_ops():
    """Register EXP_ANT_P1/P2 custom-DVE ops (rows 17/18, free on TRN2).

    exp(0.125*s) = p(s)^256 with p(s) = ((c3*s + c2)*s + c1)*s + 1 and
    ck = (0.125/256)^k / k! -- the Taylor poly of exp(s*2^-11), which for
    |0.125*s| <= 3.2 (observed score range) is exact to ~3e-5 in fp32;
    the 8 chained squarings amplify fp32 rounding by <= 256 * 2^-24.
    P1 = 6 ALU stages, P2 = 8 (both within the 8-stage DVE budget); each
    costs 1 elem/cycle/lane, so the pair is ~2.08 ns/col vs ACT 0.87."""
    import concourse.dve_ops as dve_ops
    from concourse.dve_spec import Spec, Src0, C0, C1, C2, One, lower, sq, _has_src1
    from concourse.dve_uop import DveOpSpec

    have = {op.name: op for op in dve_ops.OPS}
    if "EXP_ANT_P1" in have:
        return have["EXP_ANT_P1"], have["EXP_ANT_P2"]

    body1 = ((Src0 * C2 + C1) * Src0 + C0) * Src0 + One
    spec1 = Spec(
        body=body1,
        reference=lambda in0, in1, s0, s1, imm2: (
            ((in0.astype(np.float32) * np.float32(imm2) + np.float32(s1))
             * in0 + np.float32(s0)) * in0 + np.float32(1.0)
        ).astype(np.float32),
    )
    x = Src0
    for _ in range(8):
        x = sq(x)
    spec2 = Spec(
        body=x,
        reference=lambda in0, in1, s0, s1, imm2: (
            in0.astype(np.float32) ** 256
        ).astype(np.float32),
    )
    out = []
    for name, spec in (("EXP_ANT_P1", spec1), ("EXP_ANT_P2", spec2)):
        row = dve_ops._CUSTOM_DVE_ROW_BASE + len(dve_ops.OPS)
        dve_ops._SUB_OPCODE_FOR_NAME[name] = row
        uops = lower(spec, ver="v3")
        sha3 = DveOpSpec(
            name=name, opcode=row, uops=uops, rd1_en=_has_src1(spec)
        ).sha("v3")
        op = dve_ops.DveOp(name, spec, subdim=False, uops_sha={"v3": sha3})
        dve_ops.OPS.append(op)
        dve_ops.CUSTOM_DVE_SPECS[name] = spec
        out.append(op)
    return out[0], out[1]


_EXP_A = 0.125 / 256.0  # 2^-11, exact in fp32
_EXP_C1 = _EXP_A
_EXP_C2 = _EXP_A * _EXP_A / 2.0
_EXP_C3 = _EXP_A * _EXP_A * _EXP_A / 6.0


def build_nc(S=S_FULL):
    import concourse.bass as bass
    from concourse import bacc
    import concourse.mybir as mybir
    import concourse.tile as tile
    from concourse.masks import make_identity
    f32 = mybir.dt.float32
    f16 = mybir.dt.float16
    AF = mybir.ActivationFunctionType

    D = D_MODEL
    n_qc = S // 512     # 512-wide query chunks
    n_kc = S // 128     # 128-wide key tiles
    n_dc = D // 128     # 128-wide contraction chunks of D

    EXP_P1, EXP_P2 = _register_exp_ops()

    nc = bacc.Bacc()

    xT = nc.dram_tensor("xT", [D, S], f16, kind="ExternalInput")
    # weights arrive in SBUF-image layout with the bias as a trailing
    # column (f16 bias rounding is far below fp16 operand noise): one
    # contiguous DMA each
    wqT = nc.dram_tensor("wqT", [128, n_dc * 128 + 1], f16, kind="ExternalInput")
    wkT = nc.dram_tensor("wkT", [128, n_dc * 128 + 1], f16, kind="ExternalInput")
    wvT = nc.dram_tensor("wvT", [128, n_dc * 130 + 130], f16,
                         kind="ExternalInput")
    out = nc.dram_tensor("out", [S, 128], f32, kind="ExternalOutput")

    with tile.TileContext(nc) as tc:
        with (
            tc.tile_pool(name="consts", bufs=1) as consts,
            tc.tile_pool(name="persist", bufs=1) as persist,
        ):
            ident = consts.tile([128, 128], f16, name="ident")
            tiny = consts.tile([128, 8], f32, name="tiny")
            tiny_o = consts.tile([128, 8], f16, name="tiny_o")
            wq_sb = consts.tile([128, n_dc * 128 + 1], f16, name="wq_sb")
            wk_sb = consts.tile([128, n_dc * 128 + 1], f16, name="wk_sb")
            wv_sb = consts.tile([128, n_dc * 130 + 130], f16, name="wv_sb")
            bq_sb = consts.tile([128, 1], f32, name="bq_sb")
            bk_sb = consts.tile([128, 1], f32, name="bk_sb")
            bvb_sb = consts.tile([128, 130], f32, name="bvb_sb")
            xt = persist.tile([128, n_dc * S], f16, name="xt")

            def xs(dc, sl):
                return xt[:, dc * S + sl.start: dc * S + sl.stop]
            qt = persist.tile([128, S], f16, name="qt")
            kt = persist.tile([128, S], f16, name="kt")
            # V1[kc*130 + h*65 : +65] = [V_h | ones] per key tile kc.
            v1 = persist.tile([128, n_kc * 130], f16, name="v1")

            # ACT table preload: dummy exp at t=0 hides the ~2.7us load.
            nc.vector.memset(tiny[:], 0.0)
            nc.scalar.activation(tiny_o[:], tiny[:], AF.Exp, scale=0.125)
            make_identity(nc, ident)

            # DMAs.  Issue order IS priority: each dma_start costs
            # 0.6-1.4us of Sync issue time and the DMA engines round-robin
            # fairly over everything outstanding, so: tiny weights first,
            # then x block0 as a single 3D transfer, then the later x
            # pieces (xB is also held behind xA by a deliberate 1-column
            # WAW overlap so block0+xA are never starved).
            x_src = xT[:, :].rearrange("(dc p) s -> p dc s", dc=n_dc)
            x_dst = xt[:].rearrange("p (dc s) -> p dc s", s=S)
            nc.sync.dma_start(wk_sb[:], wkT[:, :])
            nc.sync.dma_start(wq_sb[:], wqT[:, :])
            nc.sync.dma_start(x_dst[:, :, 0:512], x_src[:, :, 0:512])
            nc.sync.dma_start(x_dst[:, :, 512:1024], x_src[:, :, 512:1024])
            nc.sync.dma_start(wv_sb[:], wvT[:, :])
            nc.sync.dma_start(x_dst[:, :, 1024:2049], x_src[:, :, 1024:2049])
            nc.sync.dma_start(x_dst[:, :, 2048:S], x_src[:, :, 2048:S])
            # biases ride in the weight images as f16; widen to f32 once
            nc.vector.tensor_copy(bk_sb[:], wk_sb[:, n_dc * 128: n_dc * 128 + 1])
            nc.vector.tensor_copy(bq_sb[:], wq_sb[:, n_dc * 128: n_dc * 128 + 1])
            nc.vector.tensor_copy(bvb_sb[:], wv_sb[:, n_dc * 130: n_dc * 130 + 130])

            with (
                tc.tile_pool(name="etp", bufs=34) as etp,
                tc.tile_pool(name="outp", bufs=2) as outp,
                tc.tile_pool(name="scrp", bufs=2) as scrp,
                tc.tile_pool(name="ps_st", bufs=2, space="PSUM") as ps_st,
            ):
                # ---------- projection pieces ----------
                def emit_kq(dst, w_sb, b_sb, b, pool):
                    cs = slice(b * 512, (b + 1) * 512)
                    p = pool.tile([128, 512], f32, name="pp", tag="pp")
                    for dc in range(n_dc):
                        nc.tensor.matmul(
                            p[:],
                            lhsT=w_sb[:, dc * 128:(dc + 1) * 128],
                            rhs=xs(dc, cs),
                            start=(dc == 0),
                            stop=(dc == n_dc - 1),
                        )
                    nc.vector.tensor_scalar_add(dst[:, cs], p[:], b_sb[:])

                vq_done = [0]

                def emit_vq(pool):
                    # V projection for the next 128-token tile.
                    st_ = vq_done[0]
                    ss = slice(st_ * 128, (st_ + 1) * 128)
                    p = pool.tile([128, 512], f32, name="pp", tag="pp")
                    for dc in range(n_dc):
                        nc.tensor.matmul(
                            p[:, 0:130],
                            lhsT=xs(dc, ss),
                            rhs=wv_sb[:, dc * 130:(dc + 1) * 130],
                            start=(dc == 0),
                            stop=(dc == n_dc - 1),
                        )
                    nc.vector.tensor_add(
                        v1[:, st_ * 130:(st_ + 1) * 130], p[:, 0:130], bvb_sb[:]
                    )
                    vq_done[0] += 1

                # ---------- attention ----------
                ev_fifo = []        # (qc, kc, h, et_tile, col_off)
                ev_left = {}        # qc -> slices not yet popped
                po_by_qc = {}

                def emit_norm(po, qc, st_pool):
                    # res[:, t*128+h*64 : +64] = head h of output rows
                    # qc*512 + t*128 + [0:128); shipped as one 3D DMA
                    res = outp.tile([128, 512], f32, name="res", tag="res")
                    last = qc == n_qc - 1
                    ots = []
                    for h in range(2):
                        ot = outp.tile([128, 512], f16, name="ot", tag="ot")
                        if last and h == 0:
                            nc.scalar.copy(ot[:], po[h][:])
                        else:
                            nc.vector.tensor_copy(ot[:], po[h][:])
                        ots.append(ot)
                    for t in range(4):
                        for h in range(2):
                            if last:
                                pt = st_pool.tile([128, 65], f16, name="pt",
                                                  tag="st")
                                nc.tensor.transpose(
                                    pt[:],
                                    ots[h][0:65, t * 128:(t + 1) * 128],
                                    ident[0:65, 0:65],
                                )
                                src = pt
                            else:
                                tp = outp.tile([128, 128], f16, name="tp",
                                               tag="tp")
                                nc.sync.dma_start_transpose(
                                    tp[:], ots[h][:, t * 128:(t + 1) * 128]
                                )
                                src = tp
                            rcp = outp.tile([128, 1], f32, name="rcp", tag="rcp")
                            nc.vector.reciprocal(rcp[:], src[:, 64:65])
                            c0 = t * 128 + h * 64
                            if last and h == 0:
                                nc.scalar.mul(
                                    res[:, c0:c0 + 64], src[:, 0:64], rcp[:],
                                )
                            else:
                                nc.vector.tensor_scalar_mul(
                                    res[:, c0:c0 + 64], src[:, 0:64], rcp[:],
                                )
                    nc.sync.dma_start(
                        out[qc * 512:(qc + 1) * 512, :]
                        .rearrange("(t p) c -> p t c", t=4),
                        res[:].rearrange("p (t c) -> p t c", t=4),
                    )

                def pop_ev(n, ps_o):
                    popped = 0
                    while ev_fifo and popped < n:
                        qc, kc, h, et, off = ev_fifo[0]
                        # never emit a pop ahead of its V tile: a blocked
                        # matmul would head-block the in-order PE queue
                        if kc + 2 > vq_done[0] and vq_done[0] < n_kc:
                            break
                        ev_fifo.pop(0)
                        if qc not in po_by_qc:
                            po_by_qc[qc] = [
                                ps_o.tile([128, 512], f32, name=f"po{h2}",
                                          tag=f"po{h2}")
                                for h2 in range(2)
                            ]
                        po = po_by_qc[qc]
                        nc.tensor.matmul(
                            po[h][0:65, :],
                            lhsT=v1[:, kc * 130 + h * 65: kc * 130 + h * 65 + 65],
                            rhs=et[:, off:off + 512],
                            start=(kc == 0),
                            stop=(kc == n_kc - 1),
                        )
                        popped += 1
                        ev_left[qc] -= 1
                        if ev_left[qc] == 0:
                            emit_norm(po_by_qc.pop(qc), qc, ps_st)

                def emit_st_exp(qc, batch, st_pool, wd_map=None):
                    if qc not in ev_left:
                        ev_left[qc] = 2 * n_kc
                    qs = slice(qc * 512, (qc + 1) * 512)
                    w = len(batch) * 512
                    st_ps = st_pool.tile([128, w], f32, name="st_ps", tag="st")
                    et = etp.tile([128, w], f16, name="et", tag="et")
                    for si, (kc, h) in enumerate(batch):
                        hp = slice(h * 64, (h + 1) * 64)
                        nc.tensor.matmul(
                            st_ps[:, si * 512:(si + 1) * 512],
                            lhsT=kt[hp, kc * 128:(kc + 1) * 128],
                            rhs=qt[hp, qs],
                            start=True,
                            stop=True,
                        )
                    # trailing wd cols on DVE (2-op exp chain), rest on ACT
                    wd = (wd_map or {}).get(w, 0)
                    wa = w - wd
                    if wa:
                        nc.scalar.activation(
                            et[:, 0:wa], st_ps[:, 0:wa], AF.Exp, scale=0.125
                        )
                    if wd:
                        sc = scrp.tile([128, wd], f32, name="sc", tag="sc")
                        nc.vector._custom_dve(
                            EXP_P1, out=sc[:], in0=st_ps[:, wa:w],
                            s0=_EXP_C1, s1=_EXP_C2, imm2=_EXP_C3,
                        )
                        nc.vector._custom_dve(EXP_P2, out=et[:, wa:w], in0=sc[:])
                    for si, (kc, h) in enumerate(batch):
                        ev_fifo.append((qc, kc, h, et, si * 512))

                def chunk_list(qc, sizes):
                    slices = [(kc, h) for kc in range(n_kc) for h in range(2)]
                    o, res = 0, []
                    for sz in sizes:
                        res.append(slices[o:o + sz])
                        o += sz
                    return res

                # 22 chunks per qc: two 1024-wide starters, then 1536-wide.
                # The qc ends on a big chunk so the next qc's S^T matmuls
                # are always covered by >= 1.3us of exp time.
                SIZES = [2, 2] + [3] * 20

                # ---- qc0/qc1: attention + pipelined projections ----
                # No E@V pops here: the projection ring owns the two PSUM
                # banks that later hold the E@V accumulators; a deep fp16
                # ring buffers all prologue exp outputs instead.
                with tc.tile_pool(name="pproj", bufs=2, space="PSUM") as pproj:
                    # ~3.4us of dummy matmuls while x block0 is in flight:
                    # trips the PE HAM clock-gate to 2.4GHz so the first
                    # real projections don't run at half clock
                    for w in range(8):
                        wp = ps_st.tile([128, 512], f32, name="warm", tag="st")
                        nc.tensor.matmul(
                            wp[:], lhsT=ident[:], rhs=wk_sb[:, 0:512],
                            start=True, stop=True,
                        )
                    emit_kq(kt, wk_sb, bk_sb, 0, pproj)
                    emit_kq(qt, wq_sb, bq_sb, 0, pproj)
                    # per-chunk piece schedule: K blocks ahead of their S^T
                    # use and behind their x DMA; Q before its q-chunk
                    qc0_kq = {2 * b - 1: f"k{b}" for b in range(1, n_qc)}
                    qc0_kq.update({13 + 2 * b: f"q{b}" for b in range(1, 5)})
                    qc1_kq = {0: "q5", 2: "q6", 4: "q7"}
                    for qc, kq, dbl in ((0, qc0_kq, ()), (1, qc1_kq, (16, 18, 20))):
                        for ci, batch in enumerate(chunk_list(qc, SIZES)):
                            emit_st_exp(qc, batch, ps_st, WD_PRO)
                            piece = kq.get(ci)
                            if piece is not None:
                                b = int(piece[1:])
                                if piece[0] == "k":
                                    emit_kq(kt, wk_sb, bk_sb, b, pproj)
                                else:
                                    emit_kq(qt, wq_sb, bq_sb, b, pproj)
                            elif not (qc == 0 and ci == 0) and vq_done[0] < n_kc:
                                emit_vq(pproj)
                                if ci in dbl and vq_done[0] < n_kc:
                                    emit_vq(pproj)

                # ---- qc2..qc7: steady state + E@V catch-up ----
                with tc.tile_pool(name="ps_o", bufs=1, space="PSUM") as ps_o:
                    for qc in range(2, n_qc):
                        qsizes = ([2, 2] + [3] * 19 + [2, 1]
                                  if qc == n_qc - 1 else SIZES)
                        for ci, batch in enumerate(chunk_list(qc, qsizes)):
                            emit_st_exp(qc, batch, ps_st, WD_STE)
                            pop_ev(3 if ci == 0 else (4 if ci == 1 else 5),
                                   ps_o)
                    pop_ev(len(ev_fifo), ps_o)
    return nc


def _shard_inputs(x, Wq, bq, Wk, bk, Wv, bv):
    """Build the 8 per-core input maps from full inputs."""
    x = np.asarray(x, dtype=np.float32)
    in_maps = []
    for c in range(N_CORES):
        b, pair = c // 4, c % 4
        rows = slice(pair * 128, (pair + 1) * 128)
        wq_s = np.asarray(Wq)[rows, :].astype(np.float32)
        wk_s = np.asarray(Wk)[rows, :].astype(np.float32)
        wv_s = np.asarray(Wv)[rows, :].astype(np.float32)
        bq_s = np.asarray(bq)[rows].astype(np.float32)
        bk_s = np.asarray(bk)[rows].astype(np.float32)
        bv_s = np.asarray(bv)[rows].astype(np.float32)

        wvT = np.zeros((D_MODEL, 130), np.float32)
        wvT[:, 0:64] = wv_s[0:64].T
        wvT[:, 65:129] = wv_s[64:128].T
        wvT = wvT.reshape(4, 128, 130).transpose(1, 0, 2).reshape(128, 520)
        wq_im = wq_s.T.reshape(4, 128, 128).transpose(1, 0, 2).reshape(128, 512)
        wk_im = wk_s.T.reshape(4, 128, 128).transpose(1, 0, 2).reshape(128, 512)
        bvb = np.zeros((128, 130), np.float32)
        bvb[:, 0:64] = bv_s[0:64]
        bvb[:, 64] = 1.0
        bvb[:, 65:129] = bv_s[64:128]
        bvb[:, 129] = 1.0
        wq_im = np.concatenate([wq_im, bq_s.reshape(128, 1)], axis=1)
        wk_im = np.concatenate([wk_im, bk_s.reshape(128, 1)], axis=1)
        wvT = np.concatenate([wvT, bvb], axis=1)

        in_maps.append({
            "xT": np.ascontiguousarray(x[c // 4].T).astype(np.float16),
            "wqT": np.ascontiguousarray(wq_im).astype(np.float16),
            "wkT": np.ascontiguousarray(wk_im).astype(np.float16),
            "wvT": wvT.astype(np.float16),
        })
    return in_maps


def _gather(results):
    B, S, D = 2, S_FULL, D_MODEL
    out = np.empty((B, S, D), np.float32)
    for c in range(N_CORES):
        b, pair = c // 4, c % 4
        out[b, :, pair * 128:(pair + 1) * 128] = results[c]["out"]
    return out


def _install_profile_hook():
    """Provide antenv.axon_hooks (missing in this image) so that
    run_bass_kernel_spmd(trace=True) can capture NTFF profiles, using the
    same ctypes path trn_boot.py would have registered."""
    import sys, types, ctypes, contextlib

    if "antenv.axon_hooks" in sys.modules:
        return
    so_path = "/opt/axon/libaxon_pjrt.so"
    mod = types.ModuleType("antenv.axon_hooks")
    state = {"hook": None}
    mod.set_axon_ntff_profile_hook = lambda h: state.__setitem__("hook", h)
    mod.get_axon_ntff_profile_hook = lambda: state["hook"]
    sys.modules["antenv.axon_hooks"] = mod
    try:
        lib = ctypes.CDLL(so_path)
        if not hasattr(lib, "axon_start_nrt_profile"):
            return
        lib.axon_start_nrt_profile.argtypes = [
            ctypes.POINTER(ctypes.c_int64), ctypes.c_size_t]
        lib.axon_start_nrt_profile.restype = ctypes.c_int64
        lib.axon_stop_nrt_profile.argtypes = [ctypes.c_char_p]
        lib.axon_stop_nrt_profile.restype = ctypes.c_int64

        @contextlib.contextmanager
        def _hook(output_dir, device_ids):
            import jax
            jax.devices()
            if device_ids:
                ids = (ctypes.c_int64 * len(device_ids))(*device_ids)
                rc = lib.axon_start_nrt_profile(ids, len(device_ids))
            else:
                rc = lib.axon_start_nrt_profile(None, 0)
            if rc != 0:
                raise RuntimeError(f"axon_start_nrt_profile rc={rc}")
            try:
                yield
            finally:
                n = lib.axon_stop_nrt_profile(str(output_dir).encode())
                print(f"profile: {n} file(s) written to {output_dir}")

        state["hook"] = _hook
    except OSError:
        pass


def kernel(x, Wq, bq, Wk, bk, Wv, bv, trace=False):
    from concourse.bass_utils import run_bass_kernel_spmd

    if trace:
        _install_profile_hook()
    if "nc" not in _cached:
        nc = build_nc(S_FULL)
        nc.finalize()
        _cached["nc"] = nc
    nc = _cached["nc"]
    in_maps = _shard_inputs(x, Wq, bq, Wk, bk, Wv, bv)
    r = run_bass_kernel_spmd(nc, in_maps, list(range(N_CORES)), trace=trace)
    _cached["last_results"] = r
    return _gather(r.results)

